# revision 2
# baseline (speedup 1.0000x reference)
"""Trainium2 Bass kernel for CustomFullyConnectedLayer (soft top-k masked linear).

out = x @ W.T where W[r, c] = A[(r-c) % n, c], A = dykstra_mask(alpha, K) * V.

The mask + W construction is O(n^2) scalar work (trivial next to the
2*B*n^2 = 275 GFLOP matmul), so it runs on host in numpy; the matmul runs
on 8 NeuronCores, data-parallel over the batch dim (1024 rows per core).

Hybrid-precision contraction: 24 of the 32 contraction chunks run in bf16,
the last 8 run as 4 fp8(e4m3) DoubleRow matmuls (2 chunks per instruction
at double rate). fp8 operands carry power-of-2 scales (x*8, W*8192); the
bf16 W part is pre-scaled by the combined 65536 so every matmul accumulates
65536*out into the same fp32 PSUM, and the PSUM->SBUF drain is a single
tensor_scalar multiply by 1/65536. Measured rel_absmax vs the fp32 oracle:
0.0166 (gate 2e-2).

Device kernel (per core), mapping: psum[b,r] += xT[c,b].T @ wT[c,r]
  - stationary = xT tile [128c, 128b], moving = wT slice [128c, 512r]
  - K-contiguous accumulation loop (24 bf16 + 4 fp8-DoubleRow per psum tile)
  - W streamed in 2-bank (1024-col) double-buffered groups
"""

import numpy as np
import ml_dtypes

import concourse.bacc as bacc
import concourse.mybir as mybir
import concourse.tile as tile
from concourse.bass_utils import run_bass_kernel_spmd

N_CORES = 8
B_FULL = 8192
C = 4096  # in_features (contraction)
R = 4096  # out_features
BS = B_FULL // N_CORES  # 1024 per-core batch shard
TOPK_L = np.float32(0.01)
NUM_ITER = 50

P = 128
CT_BF = 24           # bf16 contraction chunks
CT_F8 = 8            # fp8 contraction chunks (4 DoubleRow pairs)
C_BF = CT_BF * P     # 3072
C_F8 = CT_F8 * P     # 1024
BT = BS // P         # 8 batch tiles per core
RBANK = 512          # psum bank width (fp32)
NPAIR = CT_F8 // 2   # 4 DoubleRow matmuls per psum tile

SX = np.float32(8.0)      # fp8 scale for x   (|x| < 30 -> fits e4m3 max 240)
SW = np.float32(8192.0)   # fp8 scale for W   (|W| <= 1/64 -> max 128)
S_ALL = SX * SW           # 65536; bf16 W pre-scaled by this too

TRACE = False
LAST = {}

_NC_CACHE = {}


def _ensure_ntff_hook():
    """Bridge the NTFF-profile hook: this image's ``antenv`` lacks the
    ``axon_hooks`` module that ``run_bass_kernel_spmd(trace=True)`` expects,
    but the actual ctypes hook implementation ships in ``trn_agent_boot``.
    Also stub out the S3 artifact upload (no creds in-container)."""
    import sys
    import types

    try:
        import antenv

        if "antenv.axon_hooks" not in sys.modules:
            mod = types.ModuleType("antenv.axon_hooks")
            store = {"hook": None}
            mod.set_axon_ntff_profile_hook = lambda h: store.__setitem__("hook", h)
            mod.get_axon_ntff_profile_hook = lambda: store["hook"]
            sys.modules["antenv.axon_hooks"] = mod
            antenv.axon_hooks = mod
        from antenv.axon_hooks import (
            get_axon_ntff_profile_hook,
            set_axon_ntff_profile_hook,
        )

        if get_axon_ntff_profile_hook() is None:
            from trn_agent_boot.trn_boot import _ntff_profile_via_ctypes

            set_axon_ntff_profile_hook(
                _ntff_profile_via_ctypes("/opt/axon/libaxon_pjrt.so")
            )

        import concourse.bass_utils as bu

        bu.upload_artifacts = lambda tmpdir: f"file://{tmpdir}"
        return True
    except Exception as e:  # profiling is best-effort; execution must not break
        print(f"ntff hook setup failed: {e}")
        return False


def _dykstra_mask(alpha, k):
    """Numpy mirror of the reference's Dykstra soft top-k (same fp32 op order)."""
    y = (alpha / TOPK_L).astype(np.float32)
    n = y.shape[-1]
    z = y.copy()
    p = np.zeros_like(y)
    q = np.zeros_like(y)
    for _ in range(NUM_ITER):
        w = z + p
        z1 = w + (np.float32(k) - np.sum(w)) / np.float32(n)
        p = w - z1
        w2 = z1 + q
        z = np.clip(w2, np.float32(0.0), np.float32(1.0))
        q = w2 - z
    return z


def _build_wT(V, alpha_topk):
    """W[r, c] = A[(r-c) % n, c]  ->  W.T split into a bf16 part (first
    C_BF contraction rows, pre-scaled by S_ALL) and an fp8 part (last C_F8
    rows, scaled by SW)."""
    n = R
    A = (alpha_topk[:, None] * V).astype(np.float32)
    D = np.concatenate([A, A], axis=0)  # [2n, n]
    s0, s1 = D.strides
    # W[r, c] = D[n - c + r, c] : skewed strided view, no index arrays
    W_view = np.lib.stride_tricks.as_strided(
        D[n:], shape=(n, n), strides=(s0, s1 - s0)
    )
    WT = W_view.T  # [c, r]
    wT_bf = (np.ascontiguousarray(WT[:C_BF]) * S_ALL).astype(ml_dtypes.bfloat16)
    wT_f8 = np.clip(
        np.ascontiguousarray(WT[C_BF:]) * SW, -240.0, 240.0
    ).astype(ml_dtypes.float8_e4m3)
    return wT_bf, wT_f8


def _build_nc():
    if "nc" in _NC_CACHE:
        return _NC_CACHE["nc"]

    nc = bacc.Bacc(
        "TRN2", target_bir_lowering=False, debug=False, num_devices=N_CORES
    )
    bf16 = mybir.dt.bfloat16
    f8 = mybir.dt.float8e4
    f32 = mybir.dt.float32
    xbf_d = nc.dram_tensor("xbf", [C_BF, BS], bf16, kind="ExternalInput")
    xf8_d = nc.dram_tensor("xf8", [C_F8, BS], f8, kind="ExternalInput")
    wbf_d = nc.dram_tensor("wbf", [C_BF, R], bf16, kind="ExternalInput")
    wf8_d = nc.dram_tensor("wf8", [C_F8, R], f8, kind="ExternalInput")
    out_d = nc.dram_tensor("out", [BS, R], f32, kind="ExternalOutput")

    xbf_ap = xbf_d.rearrange("(t p) b -> p t b", p=P)
    xf8_ap = xf8_d.rearrange("(t p) b -> p t b", p=P)
    wbf_ap = wbf_d.rearrange("(t p) r -> p t r", p=P)
    wf8_ap = wf8_d.rearrange("(t p) r -> p t r", p=P)
    out_ap = out_d.rearrange("b (k r) -> b k r", k=R // RBANK)

    # ct-chunk schedules: fine-grained at the head so the first matmul can
    # start early instead of waiting for multi-MB loads; coarse after.
    FIRST_BF = [1, 1, 2, 4, 8, 8]
    STEADY_BF = [8, 8, 8]
    FIRST_F8 = [4, 4]
    STEADY_F8 = [8]
    INV_S = float(1.0 / S_ALL)

    def _chunks(sizes):
        o = 0
        for s in sizes:
            yield o, s
            o += s

    with tile.TileContext(nc) as tc:
        with (
            tc.tile_pool(name="xp", bufs=1) as xp,
            tc.tile_pool(name="wp", bufs=2) as wp,
            tc.tile_pool(name="pp", bufs=2, space="PSUM") as pp,
            tc.tile_pool(name="op", bufs=2) as op,
        ):
            # x on the ACT HWDGE queue, W on the SP queue: the two input
            # streams issue descriptors in parallel; out goes to ACT.
            x_bf = xp.tile([P, CT_BF, BS], bf16)
            x_f8 = xp.tile([P, CT_F8, BS], f8)
            for o, s in _chunks(FIRST_BF):
                nc.scalar.dma_start(
                    out=x_bf[:, o : o + s, :], in_=xbf_ap[:, o : o + s, :]
                )
            for o, s in _chunks(FIRST_F8):
                nc.scalar.dma_start(
                    out=x_f8[:, o : o + s, :], in_=xf8_ap[:, o : o + s, :]
                )
            # (start_bank, group_width, bt-interleave): the head group is
            # 1 bank wide with 4 batch-tiles interleaved so the PE has
            # work during the input-DMA ramp; steady groups are 2 banks
            # wide with pairs.
            SCHED = [(0, 1, 4), (1, 2, 2), (3, 2, 2), (5, 2, 2), (7, 1, 2)]
            for gi, (r0, gw, il) in enumerate(SCHED):
                w_bf = wp.tile([P, CT_BF, gw * RBANK], bf16, tag="w")
                w_f8 = wp.tile([P, CT_F8, gw * RBANK], f8, tag="w8")
                rsl = slice(r0 * RBANK, (r0 + gw) * RBANK)
                for o, s in _chunks(FIRST_BF if gi == 0 else STEADY_BF):
                    nc.sync.dma_start(
                        out=w_bf[:, o : o + s, :], in_=wbf_ap[:, o : o + s, rsl]
                    )
                for o, s in _chunks(FIRST_F8 if gi == 0 else STEADY_F8):
                    nc.sync.dma_start(
                        out=w_f8[:, o : o + s, :], in_=wf8_ap[:, o : o + s, rsl]
                    )
                for blk in range(BT // il):
                    ps = pp.tile([P, il, gw, RBANK], f32, tag="ps")
                    for ct in range(CT_BF):
                        for u in range(il):
                            bt = blk * il + u
                            for j in range(gw):
                                nc.tensor.matmul(
                                    ps[:, u, j, :],
                                    x_bf[:, ct, bt * P : (bt + 1) * P],
                                    w_bf[:, ct, j * RBANK : (j + 1) * RBANK],
                                    start=(ct == 0),
                                    stop=False,
                                )
                    for pr in range(NPAIR):
                        for u in range(il):
                            bt = blk * il + u
                            for j in range(gw):
                                nc.tensor.matmul(
                                    ps[:, u, j, :],
                                    x_f8[:, 2 * pr : 2 * pr + 2, bt * P : (bt + 1) * P],
                                    w_f8[:, 2 * pr : 2 * pr + 2, j * RBANK : (j + 1) * RBANK],
                                    start=False,
                                    stop=(pr == NPAIR - 1),
                                    perf_mode=mybir.MatmulPerfMode.DoubleRow,
                                )
                    for u in range(il):
                        bt = blk * il + u
                        ot = op.tile([P, gw, RBANK], f32, tag="o")
                        nc.vector.tensor_scalar_mul(ot[:], ps[:, u], INV_S)
                        nc.scalar.dma_start(
                            out=out_ap[bt * P : (bt + 1) * P, r0 : r0 + gw],
                            in_=ot[:],
                        )

    nc.compile()
    _NC_CACHE["nc"] = nc
    return nc


def kernel(x=None, V=None, alpha=None, K=None, **_unused):
    x = np.asarray(x, dtype=np.float32)
    V = np.asarray(V, dtype=np.float32)
    alpha = np.asarray(alpha, dtype=np.float32)
    k = int(np.asarray(K).item())

    mask = _dykstra_mask(alpha, k)
    wT_bf, wT_f8 = _build_wT(V, mask)

    x_bf = x[:, :C_BF].astype(ml_dtypes.bfloat16)
    x_f8 = np.clip(x[:, C_BF:] * SX, -240.0, 240.0).astype(ml_dtypes.float8_e4m3)
    in_maps = []
    for i in range(N_CORES):
        bsl = slice(i * BS, (i + 1) * BS)
        in_maps.append(
            {
                "xbf": np.ascontiguousarray(x_bf[bsl].T),  # [C_BF, BS]
                "xf8": np.ascontiguousarray(x_f8[bsl].T),  # [C_F8, BS]
                "wbf": wT_bf,
                "wf8": wT_f8,
            }
        )

    nc = _build_nc()
    trace = bool(TRACE) and _ensure_ntff_hook()
    res = run_bass_kernel_spmd(
        nc, in_maps, core_ids=list(range(N_CORES)), trace=trace
    )
    LAST["exec_time_ns"] = res.exec_time_ns
    LAST["mean_exec_time_ns"] = res.mean_exec_time_ns
    LAST["trace"] = res.instructions_and_trace
    out = np.concatenate([r["out"] for r in res.results], axis=0)
    return np.asarray(out, dtype=np.float32)


# revision 3
# speedup vs baseline: 1.0214x; 1.0214x over previous
"""Trainium2 Bass kernel for CustomFullyConnectedLayer (soft top-k masked linear).

out = x @ W.T where W[r, c] = A[(r-c) % n, c], A = dykstra_mask(alpha, K) * V.

The mask + W construction is O(n^2) scalar work (trivial next to the
2*B*n^2 = 275 GFLOP matmul), so it runs on host in numpy; the matmul runs
on 8 NeuronCores, data-parallel over the batch dim (1024 rows per core),
in bf16 with fp32 PSUM accumulation. (fp8 DoubleRow was tried and is a
net loss: its presence in the NEFF statically caps the PE clock at 2.0GHz
vs 2.4GHz, wiping out the row savings at any error-feasible split.)

Device kernel (per core), mapping: psum[b,r] += xT[c,b].T @ wT[c,r]
  - stationary = xT tile [128c, 128b], moving = wT slice [128c, 512r]
  - K-contiguous accumulation loop (all 32 c-chunks per psum tile)
  - W streamed in 2-bank (1024-col) double-buffered groups

Ramp-phase optimizations (the steady state is 100% PE-packed at 216ns
per matmul; all overhead is in the first ~45us + tail):
  - x is split into two chunk-major halves (batch tiles 0-3 / 4-7) so
    the second half streams during the head group's second block instead
    of competing with W for HBM upfront.
  - the head W group (output bank 0) is fed from a chunk-major repack so
    multi-chunk DMAs are fully contiguous per partition (>=2KB lines).
  - ~20 dummy matmuls off a memset scratch tile run during the initial
    DMA wait to walk the PE through its p-state ladder (0.65->1.2->2.4
    GHz) before the first real matmul.
  - output is written as bf16 (halves out traffic, ~0.2% extra rounding,
    host upcasts) and the last W group drains one psum bank at a time to
    shorten the tail.
"""

import numpy as np
import ml_dtypes

import concourse.bacc as bacc
import concourse.mybir as mybir
import concourse.tile as tile
from concourse.bass_utils import run_bass_kernel_spmd

N_CORES = 8
B_FULL = 8192
C = 4096  # in_features (contraction)
R = 4096  # out_features
BS = B_FULL // N_CORES  # 1024 per-core batch shard
TOPK_L = np.float32(0.01)
NUM_ITER = 50

P = 128
CT = C // P          # 32 contraction chunks
BT = BS // P         # 8 batch tiles per core
BH = BS // 2         # 512 batch cols per x half
RBANK = 512          # psum bank width (fp32)
N_WARM = 20          # PE p-state warmup matmuls

TRACE = False
LAST = {}

_NC_CACHE = {}


def _ensure_ntff_hook():
    """Bridge the NTFF-profile hook: this image's ``antenv`` lacks the
    ``axon_hooks`` module that ``run_bass_kernel_spmd(trace=True)`` expects,
    but the actual ctypes hook implementation ships in ``trn_agent_boot``.
    Also stub out the S3 artifact upload (no creds in-container)."""
    import sys
    import types

    try:
        import antenv

        if "antenv.axon_hooks" not in sys.modules:
            mod = types.ModuleType("antenv.axon_hooks")
            store = {"hook": None}
            mod.set_axon_ntff_profile_hook = lambda h: store.__setitem__("hook", h)
            mod.get_axon_ntff_profile_hook = lambda: store["hook"]
            sys.modules["antenv.axon_hooks"] = mod
            antenv.axon_hooks = mod
        from antenv.axon_hooks import (
            get_axon_ntff_profile_hook,
            set_axon_ntff_profile_hook,
        )

        if get_axon_ntff_profile_hook() is None:
            from trn_agent_boot.trn_boot import _ntff_profile_via_ctypes

            set_axon_ntff_profile_hook(
                _ntff_profile_via_ctypes("/opt/axon/libaxon_pjrt.so")
            )

        import concourse.bass_utils as bu

        bu.upload_artifacts = lambda tmpdir: f"file://{tmpdir}"
        return True
    except Exception as e:  # profiling is best-effort; execution must not break
        print(f"ntff hook setup failed: {e}")
        return False


def _dykstra_mask(alpha, k):
    """Numpy mirror of the reference's Dykstra soft top-k (same fp32 op order)."""
    y = (alpha / TOPK_L).astype(np.float32)
    n = y.shape[-1]
    z = y.copy()
    p = np.zeros_like(y)
    q = np.zeros_like(y)
    for _ in range(NUM_ITER):
        w = z + p
        z1 = w + (np.float32(k) - np.sum(w)) / np.float32(n)
        p = w - z1
        w2 = z1 + q
        z = np.clip(w2, np.float32(0.0), np.float32(1.0))
        q = w2 - z
    return z


def _chunk_major(a2d):
    """[C, cols] -> [128, CT, cols] so that s-chunk DMA slices are s*cols
    contiguous elements per partition."""
    cols = a2d.shape[1]
    return np.ascontiguousarray(
        a2d.reshape(CT, P, cols).transpose(1, 0, 2)
    )


def _build_w(V, alpha_topk):
    """W[r, c] = A[(r-c) % n, c]  ->  W.T as bf16 [c, r]; returned as
    (wh, wbf): wh = chunk-major repack of output cols 0:512 (head group),
    wbf = cols 512:4096 in plain [c, r] layout."""
    n = R
    A = (alpha_topk[:, None] * V).astype(np.float32)
    D = np.concatenate([A, A], axis=0)  # [2n, n]
    s0, s1 = D.strides
    # W[r, c] = D[n - c + r, c] : skewed strided view, no index arrays
    W_view = np.lib.stride_tricks.as_strided(
        D[n:], shape=(n, n), strides=(s0, s1 - s0)
    )
    wT = W_view.T.astype(ml_dtypes.bfloat16, order="C")  # [c, r]
    wh = _chunk_major(wT[:, :RBANK])                     # [128, 32, 512]
    wbf = np.ascontiguousarray(wT[:, RBANK:])            # [4096, 3584]
    return wh, wbf


def _build_nc():
    if "nc" in _NC_CACHE:
        return _NC_CACHE["nc"]

    nc = bacc.Bacc(
        "TRN2", target_bir_lowering=False, debug=False, num_devices=N_CORES
    )
    bf16 = mybir.dt.bfloat16
    f32 = mybir.dt.float32
    xa_d = nc.dram_tensor("xa", [P, CT, BH], bf16, kind="ExternalInput")
    xb_d = nc.dram_tensor("xb", [P, CT, BH], bf16, kind="ExternalInput")
    wh_d = nc.dram_tensor("wh", [P, CT, RBANK], bf16, kind="ExternalInput")
    wbf_d = nc.dram_tensor("wbf", [C, R - RBANK], bf16, kind="ExternalInput")
    out_d = nc.dram_tensor("out", [BS, R], bf16, kind="ExternalOutput")

    wbf_ap = wbf_d.rearrange("(t p) r -> p t r", p=P)
    out_ap = out_d.rearrange("b (k r) -> b k r", k=R // RBANK)

    def _chunks(sizes):
        o = 0
        for s in sizes:
            yield o, s
            o += s

    with tile.TileContext(nc) as tc:
        with (
            tc.tile_pool(name="xp", bufs=1) as xp,
            tc.tile_pool(name="wp", bufs=2) as wp,
            tc.tile_pool(name="pp", bufs=2, space="PSUM") as pp,
            tc.tile_pool(name="op", bufs=2) as op,
        ):
            # x on the ACT HWDGE queue, W on the SP queue: the two input
            # streams issue descriptors in parallel; out goes to ACT.
            xa_sb = xp.tile([P, CT, BH], bf16, tag="xa")
            xb_sb = xp.tile([P, CT, BH], bf16, tag="xb")
            scr = xp.tile([P, RBANK], bf16, tag="warm")
            nc.vector.memset(scr[:], 1.0)

            # first piece covers only the first stationary tile so the
            # first matmul can fire as early as possible
            nc.scalar.dma_start(out=xa_sb[:, 0:1, 0:P], in_=xa_d[:, 0:1, 0:P])
            nc.scalar.dma_start(out=xa_sb[:, 0:1, P:BH], in_=xa_d[:, 0:1, P:BH])
            for o, s in _chunks([1, 2, 4, 8, 8, 8]):
                nc.scalar.dma_start(
                    out=xa_sb[:, 1 + o : 1 + o + s, :], in_=xa_d[:, 1 + o : 1 + o + s, :]
                )
            for o, s in _chunks([8, 8, 8, 8]):
                nc.scalar.dma_start(
                    out=xb_sb[:, o : o + s, :], in_=xb_d[:, o : o + s, :]
                )

            def xt(ct, bt):
                if bt < 4:
                    return xa_sb[:, ct, bt * P : (bt + 1) * P]
                return xb_sb[:, ct, (bt - 4) * P : (bt - 3) * P]

            # (start_bank, group_width, bt-interleave): the head group is
            # 1 bank wide with 4 batch-tiles interleaved so the PE has
            # work during the input-DMA ramp; steady groups are 2 banks
            # wide with pairs; the last group drains 1 bank at a time to
            # shorten the tail.
            SCHED = [(0, 1, 4), (1, 2, 2), (3, 2, 2), (5, 2, 2), (7, 1, 1)]
            for gi, (r0, gw, il) in enumerate(SCHED):
                w_sb = wp.tile([P, CT, 2 * RBANK], bf16, tag="w")
                if gi == 0:
                    for o, s in _chunks([1, 1, 2, 4, 8, 8, 8]):
                        nc.sync.dma_start(
                            out=w_sb[:, o : o + s, 0:RBANK],
                            in_=wh_d[:, o : o + s, :],
                        )
                else:
                    rsl = slice((r0 - 1) * RBANK, (r0 - 1 + gw) * RBANK)
                    for o, s in _chunks([4, 4, 8, 8, 8]):
                        nc.sync.dma_start(
                            out=w_sb[:, o : o + s, 0 : gw * RBANK],
                            in_=wbf_ap[:, o : o + s, rsl],
                        )
                for blk in range(BT // il):
                    ps = pp.tile([P, il, gw, RBANK], f32, tag="ps")
                    if gi == 0 and blk == 0:
                        # walk the PE p-state ladder during the DMA wait
                        for _ in range(N_WARM):
                            nc.tensor.matmul(
                                ps[:, 0, 0, :],
                                scr[:, 0:P],
                                scr[:],
                                start=True,
                                stop=True,
                            )
                    for ct in range(CT):
                        for u in range(il):
                            bt = blk * il + u
                            for j in range(gw):
                                nc.tensor.matmul(
                                    ps[:, u, j, :],
                                    xt(ct, bt),
                                    w_sb[:, ct, j * RBANK : (j + 1) * RBANK],
                                    start=(ct == 0),
                                    stop=(ct == CT - 1),
                                )
                    for u in range(il):
                        bt = blk * il + u
                        ot = op.tile([P, gw, RBANK], bf16, tag="o")
                        nc.vector.tensor_copy(ot[:], ps[:, u])
                        nc.scalar.dma_start(
                            out=out_ap[bt * P : (bt + 1) * P, r0 : r0 + gw],
                            in_=ot[:],
                        )

    nc.compile()
    _NC_CACHE["nc"] = nc
    return nc


def kernel(x=None, V=None, alpha=None, K=None, **_unused):
    x = np.asarray(x, dtype=np.float32)
    V = np.asarray(V, dtype=np.float32)
    alpha = np.asarray(alpha, dtype=np.float32)
    k = int(np.asarray(K).item())

    mask = _dykstra_mask(alpha, k)
    wh, wbf = _build_w(V, mask)

    x_bf = x.astype(ml_dtypes.bfloat16)
    in_maps = []
    for i in range(N_CORES):
        xT = x_bf[i * BS : (i + 1) * BS].T  # [C, BS]
        in_maps.append(
            {
                "xa": _chunk_major(xT[:, :BH]),
                "xb": _chunk_major(xT[:, BH:]),
                "wh": wh,
                "wbf": wbf,
            }
        )

    nc = _build_nc()
    trace = bool(TRACE) and _ensure_ntff_hook()
    res = run_bass_kernel_spmd(
        nc, in_maps, core_ids=list(range(N_CORES)), trace=trace
    )
    LAST["exec_time_ns"] = res.exec_time_ns
    LAST["mean_exec_time_ns"] = res.mean_exec_time_ns
    LAST["trace"] = res.instructions_and_trace
    out = np.concatenate([r["out"] for r in res.results], axis=0)
    return np.asarray(out, dtype=np.float32)
